# revision 13
# baseline (speedup 1.0000x reference)
"""LMU kernel for Trainium2, 8-core data-parallel, bf16 matmul path.

Math (per batch b, with x[b] in [D, L] layout):
  u[b]    = relu(W_u @ x[b] + b_u)                              [1, L]
  m[b]    = H @ Toep(u[b])        (causal conv via Toeplitz)    [D, L]
  h[b]    = relu(W_h[:, :D] @ m[b] + W_h[:, D:] @ x[b] + b_h)   [D, L]
  y[b]    = BN(conv_w @ h[b] + conv_b)                          [D, L]

Host-side folds (O(params)):
  F      = (W_h[:, :D] @ H).T, row-flipped  -> single K=128 contraction
           against the (flipped) Toeplitz of u
  C'     = (inv * conv_w).T, bias' = (conv_b - mean) * inv + beta

Device strategy:
  - batch dim sharded 8 ways, params replicated
  - x repacked on host to [KC, 128, BPC*L] bf16 so every DMA is
    contiguous 1KB-per-partition lines; out written as [KC, 128, BPC*L]
    f32 and un-permuted on host
  - all matmuls bf16 (fp32 PSUM accumulation)
  - u computed as a DVE multiply-accumulate over the 6 d-chunks followed
    by a single ones-vector reduction matmul (frees ~11us of PE time)
  - Toeplitz built via a small DRAM round-trip of the padded u signal
"""

import numpy as np
import ml_dtypes

import concourse.bass as bass
import concourse.mybir as mybir
from concourse import bacc
from concourse.tile import TileContext
from concourse.bass_utils import run_bass_kernel_spmd

BF = ml_dtypes.bfloat16

B, D, L = 256, 768, 128
NCORES = 8
BPC = B // NCORES          # batches per core
NB = 4                     # batches per column block
NCB = BPC // NB            # column blocks per core
NCOL = NB * L              # 512 columns per block
KC = D // 128              # 6 chunks of 128 over the D dim
THETA = 128.0
BN_EPS = 1e-5

TRACE = False
LAST_EXEC_NS = None

_H_CACHE = None
_NC_CACHE = None


def _impulse_response():
    """Replicates the reference's H = impulse response [D, L], on CPU."""
    global _H_CACHE
    if _H_CACHE is not None:
        return _H_CACHE
    import jax
    import jax.numpy as jnp
    from jax.scipy.linalg import expm

    cpu = jax.devices("cpu")[0]
    with jax.default_device(cpu):
        Q = np.arange(D, dtype=np.float32)
        R = ((2.0 * Q + 1.0) / THETA)[:, None]
        i, j = np.meshgrid(Q, Q, indexing="ij")
        A = (np.where(i < j, -1.0, (-1.0) ** (i - j + 1)).astype(np.float32)) * R
        Bm = (((-1.0) ** Q)[:, None]).astype(np.float32) * R
        Maug = np.zeros((D + 1, D + 1), dtype=np.float32)
        Maug[:D, :D] = A
        Maug[:D, D:] = Bm
        E = expm(jnp.asarray(Maug))
        Ad = E[:D, :D]
        Bd = E[:D, D:]

        def step(Apow, _):
            return Ad @ Apow, (Apow @ Bd)[:, 0]

        _, H = jax.lax.scan(step, jnp.eye(D, dtype=jnp.float32), None, length=L)
        _H_CACHE = np.asarray(H).T.astype(np.float32)  # [D, L]
    return _H_CACHE


def _build_nc():
    """Builds the (static) 8-core SPMD Bass program."""
    f32 = mybir.dt.float32
    bf16 = mybir.dt.bfloat16
    nc = bacc.Bacc("TRN2", target_bir_lowering=False, debug=False, num_devices=NCORES)

    x_d = nc.dram_tensor("xh", [KC, 128, BPC * L], bf16, kind="ExternalInput").ap()
    # whx/ctm are stored j-major: [j, i, p, c] so one DMA fetches the full
    # K=768 column block that output chunk j needs
    whx_d = nc.dram_tensor("whx", [KC, KC, 128, 128], bf16, kind="ExternalInput").ap()
    ct_d = nc.dram_tensor("ctm", [KC, KC, 128, 128], bf16, kind="ExternalInput").ap()
    f_d = nc.dram_tensor("fmat", [128, D], bf16, kind="ExternalInput").ap()
    wu_d = nc.dram_tensor("wu", [128, KC], f32, kind="ExternalInput").ap()
    vecs_d = nc.dram_tensor("vecs", [128, KC, 3], f32, kind="ExternalInput").ap()
    out_d = nc.dram_tensor("out", [KC, 128, BPC * L], f32, kind="ExternalOutput").ap()
    upad_d = nc.dram_tensor("upad", [BPC * 2 * L], bf16).ap()  # internal scratch

    PSTR = BPC * L  # partition stride of xh / out in DRAM elements

    with TileContext(nc) as tc:
        with (
            tc.tile_pool(name="const", bufs=1) as const,
            tc.tile_pool(name="xpool", bufs=18) as xpool,
            tc.tile_pool(name="zpool", bufs=3) as zpool,
            tc.tile_pool(name="hpool", bufs=12) as hpool,
            tc.tile_pool(name="tpool", bufs=3) as tpool,
            tc.tile_pool(name="opool", bufs=8) as opool,
            tc.tile_pool(name="upool", bufs=2) as upool,
            tc.tile_pool(name="ps", bufs=6, space="PSUM") as ps,
            tc.tile_pool(name="pur", bufs=2, space="PSUM") as pur,
        ):
            # ---- constant tiles (direct bf16 DMA, no staging copies) ----
            whx_sb = const.tile([128, KC, D], bf16)
            ct_sb = const.tile([128, KC, D], bf16)
            f_sb = const.tile([128, D], bf16)
            wu_sb = const.tile([128, KC], f32)
            vecs_sb = const.tile([128, KC, 3], f32)
            ones_sb = const.tile([128, 1], bf16)
            zt = const.tile([128, 2 * BPC], bf16)

            def load_wj(dram, j, eng):
                """One DMA: column block j of whx/ct for all 6 K-chunks."""
                eng.dma_start(
                    out=(whx_sb if dram is whx_d else ct_sb)[:, :, j * 128:(j + 1) * 128],
                    in_=bass.AP(
                        tensor=dram.tensor,
                        offset=j * KC * 128 * 128,
                        ap=[[128, 128], [128 * 128, KC], [1, 128]],
                    ),
                )

            # j0/j1 ride the earliest-online queues so the PE can start ASAP
            load_wj(whx_d, 0, nc.sync)
            load_wj(whx_d, 1, nc.scalar)

            def load_x(cb, engs=(nc.sync, nc.scalar)):
                """DMA the 6 bf16 x chunk tiles for column block cb."""
                xr = []
                for i in range(KC):
                    xt = xpool.tile([128, NCOL], bf16, tag="xt")
                    engs[i % len(engs)].dma_start(
                        out=xt[:],
                        in_=bass.AP(
                            tensor=x_d.tensor,
                            offset=i * 128 * PSTR + cb * NCOL,
                            ap=[[PSTR, 128], [1, NCOL]],
                        ),
                    )
                    xr.append(xt)
                return xr

            def z_chain(xr):
                """DVE MAC over the 6 d-chunks: z = sum_i wu_i * x_i."""
                zp = zpool.tile([128, NCOL], bf16, tag="z0")
                nc.vector.tensor_scalar_mul(zp[:], xr[0][:], wu_sb[:, 0:1])
                for i in range(1, KC):
                    zn = zpool.tile([128, NCOL], bf16, tag=f"z{i % 2}")
                    nc.vector.scalar_tensor_tensor(
                        out=zn[:], in0=xr[i][:], scalar=wu_sb[:, i:i + 1],
                        in1=zp[:], op0=mybir.AluOpType.mult,
                        op1=mybir.AluOpType.add,
                    )
                    zp = zn
                return zp

            def u_finish(cb, zp):
                """ones reduce-MM -> relu -> Toeplitz via upad round-trip."""
                psu = pur.tile([1, NCOL], f32, tag="pu")
                nc.tensor.matmul(psu[:], ones_sb[:], zp[:], start=True, stop=True)
                u_sb = upool.tile([1, NCOL], bf16, tag="u")
                nc.scalar.activation(u_sb[:], psu[:],
                                     mybir.ActivationFunctionType.Relu,
                                     bias=vecs_sb[0:1, 0, 2:3])
                nc.scalar.dma_start(
                    out=bass.AP(tensor=upad_d.tensor,
                                offset=cb * NB * 2 * L + L,
                                ap=[[2 * L, NB], [1, L]]),
                    in_=u_sb[:],
                )
                t_r = tpool.tile([128, NCOL], bf16, tag="tr")
                nc.scalar.dma_start(
                    out=t_r[:],
                    in_=bass.AP(tensor=upad_d.tensor,
                                offset=cb * NB * 2 * L + 1,
                                ap=[[1, 128], [2 * L, NB], [1, L]]),
                )
                return t_r

            def step3_mm_x(ps3, j, xr, start):
                for i in range(KC):
                    nc.tensor.matmul(ps3[:], whx_sb[:, i, j * 128:(j + 1) * 128],
                                     xr[i][:], start=(start and i == 0), stop=False,
                                     skip_group_check=True)

            def step3_mm_t(ps3, j, t_r):
                nc.tensor.matmul(ps3[:], f_sb[:, j * 128:(j + 1) * 128], t_r[:],
                                 start=False, stop=True, skip_group_check=True)

            def relu_h(ps3, j):
                hj = hpool.tile([128, NCOL], bf16, tag="h")
                nc.scalar.activation(hj[:], ps3[:],
                                     mybir.ActivationFunctionType.Relu,
                                     bias=vecs_sb[:, j, 0:1])
                return hj

            OUT_ENGS = (nc.sync, nc.gpsimd, nc.scalar)

            def step4(cb, hs):
                for j in range(KC):
                    ps4 = ps.tile([128, NCOL], f32, tag="ps")
                    for i in range(KC):
                        nc.tensor.matmul(ps4[:], ct_sb[:, i, j * 128:(j + 1) * 128],
                                         hs[i][:], start=(i == 0), stop=(i == KC - 1),
                                         skip_group_check=True)
                    oj = opool.tile([128, NCOL], f32, tag="o")
                    nc.vector.tensor_scalar_add(oj[:], ps4[:], vecs_sb[:, j, 1:2])
                    OUT_ENGS[j % 3].dma_start(
                        out=bass.AP(
                            tensor=out_d.tensor,
                            offset=j * 128 * PSTR + cb * NCOL,
                            ap=[[PSTR, 128], [1, NCOL]],
                        ),
                        in_=oj[:],
                    )

            # ---- block 0: emit all 36 x-MMs before the t-MMs so the PE has
            # ~8us of runway while the first u / Toeplitz resolves; the
            # reduce-MM for u(0) goes after j=2 so the round-trip hides ----
            xr_cur = load_x(0)
            nc.sync.dma_start(out=wu_sb[:], in_=wu_d)
            load_wj(whx_d, 2, nc.sync)
            load_wj(whx_d, 3, nc.scalar)
            load_wj(whx_d, 4, nc.gpsimd)
            load_wj(whx_d, 5, nc.gpsimd)
            nc.scalar.dma_start(out=vecs_sb[:], in_=vecs_d)
            nc.sync.dma_start(out=f_sb[:], in_=f_d)
            nc.vector.memset(ones_sb[:], 1.0)
            nc.vector.memset(zt[:], 0.0)
            nc.gpsimd.dma_start(
                out=bass.AP(tensor=upad_d.tensor, offset=0,
                            ap=[[1, BPC * 2 * L]]),
                in_=zt[:],
            )
            z_cur = z_chain(xr_cur)
            xr_next = load_x(1)
            ps3s = []
            for j in range(KC):
                ps3 = ps.tile([128, NCOL], f32, tag="ps")
                step3_mm_x(ps3, j, xr_cur, start=True)
                ps3s.append(ps3)
                if j == 2:
                    t_cur = u_finish(0, z_cur)
            # ct loads overlap the first MM stream (no PE dependency yet)
            for j in range(KC):
                load_wj(ct_d, j, nc.gpsimd)
            z_next = z_chain(xr_next)
            hs = []
            for j in range(KC):
                step3_mm_t(ps3s[j], j, t_cur)
                hs.append(relu_h(ps3s[j], j))
            t_next = u_finish(1, z_next)
            xr_fut = load_x(2)
            step4(0, hs)
            xr_cur, xr_next = xr_next, xr_fut
            t_cur = t_next

            # ---- steady-state blocks ----
            for cb in range(1, NCB):
                hs = []
                for j in range(KC):
                    ps3 = ps.tile([128, NCOL], f32, tag="ps")
                    step3_mm_x(ps3, j, xr_cur, start=True)
                    step3_mm_t(ps3, j, t_cur)
                    hs.append(relu_h(ps3, j))
                if cb + 1 < NCB:
                    z_next = z_chain(xr_next)
                    t_next = u_finish(cb + 1, z_next)
                    xr_fut = load_x(cb + 2) if cb + 2 < NCB else None
                else:
                    t_next = xr_fut = None
                step4(cb, hs)
                xr_cur, xr_next, t_cur = xr_next, xr_fut, t_next

    if not nc.is_finalized():
        nc.finalize()
    return nc


def _get_nc():
    global _NC_CACHE
    if _NC_CACHE is None:
        _NC_CACHE = _build_nc()
    return _NC_CACHE


def _ensure_ntff_hook():
    """Register the NTFF profile hook if the deployment lacks antenv.axon_hooks."""
    import sys
    import types
    try:
        from antenv.axon_hooks import get_axon_ntff_profile_hook  # noqa: F401
        return
    except ImportError:
        pass
    try:
        from trn_agent_boot.trn_boot import _ntff_profile_via_ctypes
        hook = _ntff_profile_via_ctypes("/opt/axon/libaxon_pjrt.so")
        mod = types.ModuleType("antenv.axon_hooks")
        mod.get_axon_ntff_profile_hook = lambda: hook
        mod.set_axon_ntff_profile_hook = lambda h: None
        import antenv
        sys.modules["antenv.axon_hooks"] = mod
        antenv.axon_hooks = mod
    except Exception:
        pass


def kernel(x, W_u, b_u, W_h, b_h, conv_w, conv_b, bn_gamma, bn_beta, bn_mean,
           bn_var):
    global LAST_EXEC_NS
    x = np.asarray(x, dtype=np.float32)
    W_u = np.asarray(W_u, dtype=np.float64)
    b_u = np.asarray(b_u, dtype=np.float64)
    W_h = np.asarray(W_h, dtype=np.float64)
    b_h = np.asarray(b_h, dtype=np.float64)
    conv_w = np.asarray(conv_w, dtype=np.float64)
    conv_b = np.asarray(conv_b, dtype=np.float64)
    bn_gamma = np.asarray(bn_gamma, dtype=np.float64)
    bn_beta = np.asarray(bn_beta, dtype=np.float64)
    bn_mean = np.asarray(bn_mean, dtype=np.float64)
    bn_var = np.asarray(bn_var, dtype=np.float64)
    assert x.shape == (B, D, L)

    H = _impulse_response().astype(np.float64)  # [D, L]

    # host folds (O(params) only)
    F = (W_h[:, :D] @ H).T[::-1, :]                      # [L, D], row-flipped
    whx = W_h[:, D:].T.reshape(KC, 128, KC, 128).transpose(2, 0, 1, 3)  # [j,i,p,c]
    inv = bn_gamma / np.sqrt(bn_var + BN_EPS)
    ctm = (conv_w[:, :, 0] * inv[:, None]).T.reshape(KC, 128, KC, 128).transpose(2, 0, 1, 3)
    bias2 = (conv_b - bn_mean) * inv + bn_beta
    vecs = np.stack(
        [b_h.reshape(KC, 128).T, bias2.reshape(KC, 128).T,
         np.full((128, KC), b_u[0])], axis=2
    ).astype(np.float32)                                 # [128, KC, 3]

    # repack x: [B, D, L] f32 -> per-core [KC, 128, BPC*L] bf16
    xq = x.reshape(NCORES, BPC, KC, 128, L).transpose(0, 2, 3, 1, 4).astype(BF)
    xq = np.ascontiguousarray(xq).reshape(NCORES, KC, 128, BPC * L)

    nc = _get_nc()
    shared = {
        "whx": np.ascontiguousarray(whx).astype(BF),
        "ctm": np.ascontiguousarray(ctm).astype(BF),
        "fmat": np.ascontiguousarray(F).astype(BF),
        "wu": np.ascontiguousarray(W_u[0].reshape(KC, 128).T).astype(np.float32),
        "vecs": vecs,
    }
    in_maps = []
    for c in range(NCORES):
        m = dict(shared)
        m["xh"] = xq[c]
        in_maps.append(m)

    if TRACE:
        _ensure_ntff_hook()
    res = run_bass_kernel_spmd(nc, in_maps, list(range(NCORES)), trace=TRACE)
    LAST_EXEC_NS = res.exec_time_ns
    out = np.stack([
        res.results[c]["out"].reshape(KC, 128, BPC, L) for c in range(NCORES)
    ])  # [NCORES, KC, 128, BPC, L]
    out = out.transpose(0, 3, 1, 2, 4).reshape(B, D, L)
    return np.ascontiguousarray(out)
